# revision 4
# baseline (speedup 1.0000x reference)
"""ChaosAttention Trainium2 kernel.

Problem: B=2, L=2048, D=1024, H=16 heads (hd=64), chaos-gated attention.

Sharding (8 NeuronCores): data-parallel over B (2) x tensor-parallel over
head groups (4 groups of 4 heads). Core c handles batch b=c//4, head group
g=c%4 (global heads 4g..4g+3). q/k/v/chaos projections are column-sharded,
out_proj row-sharded; per-core partial outputs are summed on host.

Device math (per core, all matmuls in float32r = full-rate fp32):
  qT[f,l] = (Wq_g)^T x^T        (feature-major so no transposes on device)
  kT[f,l] likewise (weights duplicated so kT lands in both PSUM halves)
  K'_h = [kT_h ; gate(l)*kT_h]  (128-row contraction block)
  Q'_h = [qT_h ; 0.1*cqT_h]     (cq = chaos features, host-computed)
  S^T[key,q] = K'_h^T-contract-Q'_h  -> scores + 0.1*gate[key]*chaos_scores
  P = exp(S^T/8)                (no max-subtraction needed; |S/8| < ~3)
  out_u^T = [v_h | 1]^T @ P     (ones column gives softmax denominator)
  outT_h = out_u^T * (1/denom)  (reciprocal + PE ones-matmul broadcast)
  partial = outT^T @ Wo_g       (K=64 per head, accumulated over 4 heads)

Host does only O(B*L*D) glue: x transpose, Lorenz chaos field, gate,
chaos-feature slices, weight slicing/duplication, final 4-way partial sum
(+ bo + bv@Wo which are exact row-parallel bias corrections).
"""

import sys

if "/opt/trn_rl_repo" not in sys.path:
    sys.path.insert(0, "/opt/trn_rl_repo")

import numpy as np

import concourse.bacc as bacc
import concourse.mybir as mybir
import concourse.tile as tile
from concourse.bass_utils import run_bass_kernel_spmd

# Problem constants (hardcoded per contract)
B, L, D = 2, 2048, 1024
H, HD = 16, 64
H4 = 4                  # heads per core
DG = H4 * HD            # 256 = head-group width
KB = D // 128           # 8 contraction blocks
CHAOS_STRENGTH = np.float32(0.1)
SIGMA, RHO, BETA, DT = 10.0, 28.0, 8.0 / 3.0, 0.01
N_LORENZ_STEPS = 10
SCALE = 1.0 / 8.0       # 1/sqrt(HD)

F32 = mybir.dt.float32
F32R = mybir.dt.float32r

_CACHED = {}


def _build_nc():
    nc = bacc.Bacc()

    xT = nc.dram_tensor("xT", [D, L], F32R, kind="ExternalInput")
    wq = nc.dram_tensor("wq", [D, DG], F32R, kind="ExternalInput")
    wkd = nc.dram_tensor("wkd", [D, 2 * DG], F32R, kind="ExternalInput")
    wv = nc.dram_tensor("wv", [D, DG], F32R, kind="ExternalInput")
    wo = nc.dram_tensor("wo", [DG, D], F32R, kind="ExternalInput")
    cqt = nc.dram_tensor("cqt", [DG, L], F32R, kind="ExternalInput")
    gateB = nc.dram_tensor("gateB", [128, L], F32, kind="ExternalInput")
    bqv = nc.dram_tensor("bqv", [HD, H4], F32, kind="ExternalInput")
    bkv = nc.dram_tensor("bkv", [128, H4], F32, kind="ExternalInput")
    onesd = nc.dram_tensor("onesd", [128, HD], F32R, kind="ExternalInput")
    out = nc.dram_tensor("out", [L, D], F32, kind="ExternalOutput")

    Id = mybir.ActivationFunctionType.Identity
    Exp = mybir.ActivationFunctionType.Exp
    MUL = mybir.AluOpType.mult
    ADD = mybir.AluOpType.add

    with tile.TileContext(nc) as tc:
        with (
            tc.tile_pool(name="persist", bufs=1) as pp,
            tc.tile_pool(name="mmps", bufs=2, space="PSUM") as mmps,
            tc.tile_pool(name="scps", bufs=2, space="PSUM") as scps,
            tc.tile_pool(name="avps", bufs=2, space="PSUM") as avps,
        ):
            qp = [pp.tile([128, L], F32R, tag=f"qp{h}", name=f"qp{h}") for h in range(H4)]
            kp = [pp.tile([128, L], F32R, tag=f"kp{h}", name=f"kp{h}") for h in range(H4)]
            v1 = pp.tile([128, 16, 65 * H4], F32R, tag="v1")
            gb = pp.tile([128, L], F32, tag="gb")
            ones_t = pp.tile([128, HD], F32R, tag="ones")
            bq_sb = pp.tile([HD, H4], F32, tag="bq")
            bk_sb = pp.tile([128, H4], F32, tag="bk")

            # ones columns of v1 (softmax denominator) + broadcast weights,
            # DMA-loaded: walrus rejects DVE memset to float32r tiles
            nc.sync.dma_start(
                out=v1.rearrange("p t (h e) -> p t h e", e=65)[:, :, :, 64],
                in_=onesd[:, 0:64].rearrange("p (t h) -> p t h", t=16),
            )
            nc.sync.dma_start(out=ones_t[:], in_=onesd[:, 0:HD])
            nc.sync.dma_start(out=gb[:], in_=gateB[:])
            nc.sync.dma_start(out=bq_sb[:], in_=bqv[:])
            nc.sync.dma_start(out=bk_sb[:], in_=bkv[:])
            for h in range(H4):
                # chaos-feature rows (pre-scaled by 0.1) fill Q' lower half
                nc.sync.dma_start(
                    out=qp[h][64:128, :], in_=cqt[64 * h:64 * h + 64, :]
                )

            # ---------------- phase 1: projections ----------------
            with tc.tile_pool(name="ph1", bufs=1) as p1:
                xt = p1.tile([128, KB, L], F32R, tag="xt")
                wq_sb = p1.tile([128, KB, DG], F32R, tag="wq")
                wkd_sb = p1.tile([128, KB, 2 * DG], F32R, tag="wkd")
                wv_sb = p1.tile([128, KB, DG], F32R, tag="wv")
                for kb in range(KB):
                    r = slice(kb * 128, kb * 128 + 128)
                    nc.sync.dma_start(out=xt[:, kb, :], in_=xT[r, :])
                    nc.sync.dma_start(out=wq_sb[:, kb, :], in_=wq[r, :])
                    nc.sync.dma_start(out=wkd_sb[:, kb, :], in_=wkd[r, :])
                    nc.sync.dma_start(out=wv_sb[:, kb, :], in_=wv[r, :])

                # qT (M=64 per head) and kT (M=128, duplicated weights)
                for h in range(H4):
                    for lc in range(4):
                        cs = slice(lc * 512, lc * 512 + 512)
                        ps = mmps.tile([128, 512], F32, tag="mm")
                        for kb in range(KB):
                            nc.tensor.matmul(
                                ps[0:64, :],
                                wq_sb[:, kb, 64 * h:64 * h + 64],
                                xt[:, kb, cs],
                                start=(kb == 0),
                                stop=(kb == KB - 1),
                            )
                        nc.scalar.activation(
                            qp[h][0:64, cs], ps[0:64, :], Id,
                            bias=bq_sb[:, h:h + 1], scale=1.0,
                        )
                for h in range(H4):
                    for lc in range(4):
                        cs = slice(lc * 512, lc * 512 + 512)
                        ps = mmps.tile([128, 512], F32, tag="mm")
                        for kb in range(KB):
                            nc.tensor.matmul(
                                ps[:],
                                wkd_sb[:, kb, 128 * h:128 * h + 128],
                                xt[:, kb, cs],
                                start=(kb == 0),
                                stop=(kb == KB - 1),
                            )
                        nc.scalar.activation(
                            kp[h][0:64, cs], ps[0:64, :], Id,
                            bias=bk_sb[0:64, h:h + 1], scale=1.0,
                        )
                        # gate-weighted half: (kT + bk) * gate[l]
                        nc.vector.scalar_tensor_tensor(
                            kp[h][64:128, cs],
                            in0=ps[64:128, :],
                            scalar=bk_sb[64:128, h:h + 1],
                            in1=gb[64:128, cs],
                            op0=ADD,
                            op1=MUL,
                        )
                # v in natural layout, interleaved with ones columns
                for lt in range(16):
                    ps = mmps.tile([128, 512], F32, tag="mm")
                    for kb in range(KB):
                        nc.tensor.matmul(
                            ps[:, 0:DG],
                            xt[:, kb, lt * 128:lt * 128 + 128],
                            wv_sb[:, kb, :],
                            start=(kb == 0),
                            stop=(kb == KB - 1),
                        )
                    nc.vector.tensor_copy(
                        v1[:, lt, :]
                        .rearrange("p (h d) -> p h d", d=65)[:, :, 0:64],
                        ps[:, 0:DG].rearrange("p (h d) -> p h d", d=64),
                    )

            # ---------------- phase 2: attention ----------------
            with tc.tile_pool(name="ph2", bufs=1) as p2, \
                 tc.tile_pool(name="expp", bufs=3) as expp, \
                 tc.tile_pool(name="rcpp", bufs=2) as rcpp, \
                 tc.tile_pool(name="fsb", bufs=3) as fsbp:
                ot = [p2.tile([64, L], F32R, tag=f"ot{h}", name=f"ot{h}") for h in range(H4)]
                wo_sb = p2.tile([64, H4, D], F32R, tag="wo")
                for h in range(H4):
                    nc.sync.dma_start(
                        out=wo_sb[:, h, :], in_=wo[64 * h:64 * h + 64, :]
                    )

                for h in range(H4):
                    for qc in range(4):
                        qs = slice(qc * 512, qc * 512 + 512)
                        aps = avps.tile([128, 512], F32, tag="av")
                        for g in range(8):  # pairs of key tiles
                            sps = scps.tile([128, 1024], F32, tag="sc")
                            for j in range(2):
                                kt = 2 * g + j
                                nc.tensor.matmul(
                                    sps[:, j * 512:j * 512 + 512],
                                    kp[h][:, kt * 128:kt * 128 + 128],
                                    qp[h][:, qs],
                                    start=True,
                                    stop=True,
                                )
                            et = expp.tile([128, 1024], F32R, tag="et")
                            nc.scalar.activation(et[:], sps[:], Exp, scale=SCALE)
                            for j in range(2):
                                kt = 2 * g + j
                                nc.tensor.matmul(
                                    aps[0:65, :],
                                    v1[:, kt, 65 * h:65 * h + 65],
                                    et[:, j * 512:j * 512 + 512],
                                    start=(kt == 0),
                                    stop=(kt == 15),
                                    skip_group_check=True,
                                )
                        rcp = rcpp.tile([128, 512], F32R, tag="rcp")
                        with nc.allow_low_precision(
                            "f32r reciprocal feeds a K=1 broadcast matmul; "
                            "f32r rounding is ~1e-4 relative"
                        ):
                            nc.vector.reciprocal(rcp[64:65, :], aps[64:65, :])
                        psb = mmps.tile([128, 512], F32, tag="mm")
                        nc.tensor.matmul(
                            psb[0:64, :],
                            ones_t[64:65, 0:64],
                            rcp[64:65, :],
                            start=True,
                            stop=True,
                        )
                        rcb = rcpp.tile([128, 512], F32, tag="rcb")
                        nc.vector.tensor_copy(rcb[0:64, :], psb[0:64, :])
                        nc.vector.scalar_tensor_tensor(
                            ot[h][:, qs],
                            in0=aps[0:64, :],
                            scalar=1.0,
                            in1=rcb[0:64, :],
                            op0=MUL,
                            op1=MUL,
                        )

                # ---------------- phase 3: output projection ----------------
                for lt in range(16):
                    fs = fsbp.tile([128, D], F32, tag="fs")
                    for nch in range(2):
                        fp = mmps.tile([128, 512], F32, tag="mm")
                        for h in range(H4):
                            nc.tensor.matmul(
                                fp[:],
                                ot[h][:, lt * 128:lt * 128 + 128],
                                wo_sb[:, h, nch * 512:nch * 512 + 512],
                                start=(h == 0),
                                stop=(h == H4 - 1),
                            )
                        nc.vector.tensor_copy(fs[:, nch * 512:nch * 512 + 512], fp[:])
                    nc.sync.dma_start(
                        out=out[lt * 128:lt * 128 + 128, :], in_=fs[:]
                    )

    nc.finalize()
    return nc


def _chaos_field(ci):
    """Replicates reference _chaos_field in float32 numpy."""
    xv = ci[..., 0].astype(np.float32)
    yv = ci[..., 1].astype(np.float32)
    zv = ci[..., 2].astype(np.float32)
    sigma = np.float32(SIGMA)
    rho = np.float32(RHO)
    beta = np.float32(BETA)
    dt = np.float32(DT)
    acc = np.zeros(ci.shape, dtype=np.float32)
    for _ in range(N_LORENZ_STEPS):
        dx = sigma * (yv - xv)
        dy = xv * (rho - zv) - yv
        dz = xv * yv - beta * zv
        xv = xv + dt * dx
        yv = yv + dt * dy
        zv = zv + dt * dz
        acc = acc + np.stack([xv, yv, zv], axis=-1)
    return acc / np.float32(N_LORENZ_STEPS)


def _prepare_in_maps(x, chaos_init, Wq, bq, Wk, bk, Wv, Wc, bc, Wg, bg):
    cf = _chaos_field(np.asarray(chaos_init, dtype=np.float32))  # [B,L,3]
    # gate = sigmoid(cf @ (Wc@Wg) + bc@Wg + bg), folded over the tiny K=3
    cfeat = cf @ Wc + bc                                        # [B,L,D]
    gate_logit = cfeat @ Wg + bg                                # [B,L,1]
    gate = (1.0 / (1.0 + np.exp(-gate_logit[..., 0]))).astype(np.float32)
    cq = (CHAOS_STRENGTH * cfeat).astype(np.float32)            # [B,L,D]

    in_maps = []
    for c in range(8):
        b, g = c // 4, c % 4
        gsl = slice(DG * g, DG * g + DG)
        wk_g = Wk[:, gsl]
        wkd = np.empty((D, 2 * DG), dtype=np.float32)
        for h in range(H4):
            wkd[:, 128 * h:128 * h + 64] = wk_g[:, 64 * h:64 * h + 64]
            wkd[:, 128 * h + 64:128 * h + 128] = wk_g[:, 64 * h:64 * h + 64]
        bk_g = bk[gsl].astype(np.float32)
        bkv = np.empty((128, H4), dtype=np.float32)
        for h in range(H4):
            bkv[0:64, h] = bk_g[64 * h:64 * h + 64]
            bkv[64:128, h] = bk_g[64 * h:64 * h + 64]
        in_maps.append({
            "xT": np.ascontiguousarray(x[b].T),
            "wq": np.ascontiguousarray(Wq[:, gsl]),
            "wkd": wkd,
            "wv": np.ascontiguousarray(Wv[:, gsl]),
            "wo": None,  # filled below (needs Wo)
            "cqt": np.ascontiguousarray(cq[b][:, gsl].T),
            "gateB": np.ascontiguousarray(
                np.broadcast_to(gate[b], (128, L))
            ),
            "bqv": np.ascontiguousarray(
                bq[gsl].astype(np.float32).reshape(H4, HD).T
            ),
            "bkv": bkv,
            "onesd": np.ones((128, HD), dtype=np.float32),
        })
    return in_maps


def kernel(x, mask, chaos_init, Wq, bq, Wk, bk, Wv, bv, Wo, bo, Wc, bc, Wg, bg):
    x = np.asarray(x, dtype=np.float32)
    Wq, Wk, Wv, Wo = (np.asarray(a, dtype=np.float32) for a in (Wq, Wk, Wv, Wo))
    Wc, Wg = np.asarray(Wc, np.float32), np.asarray(Wg, np.float32)
    bq, bk, bv, bo = (np.asarray(a, np.float32) for a in (bq, bk, bv, bo))
    bc, bg = np.asarray(bc, np.float32), np.asarray(bg, np.float32)

    if "nc" not in _CACHED:
        _CACHED["nc"] = _build_nc()
    nc = _CACHED["nc"]

    in_maps = _prepare_in_maps(x, chaos_init, Wq, bq, Wk, bk, Wv, Wc, bc, Wg, bg)
    for c in range(8):
        g = c % 4
        in_maps[c]["wo"] = np.ascontiguousarray(Wo[DG * g:DG * g + DG, :])

    res = run_bass_kernel_spmd(nc, in_maps, list(range(8)))

    # host unshard: sum row-parallel partials per batch, add bias terms
    bias_row = (bv @ Wo + bo).astype(np.float32)                # [D]
    out = np.empty((B, L, D), dtype=np.float32)
    for b in range(B):
        acc = res.results[4 * b + 0]["out"].astype(np.float32).copy()
        for g in range(1, 4):
            acc += res.results[4 * b + g]["out"]
        out[b] = acc + bias_row
    return out
